# revision 6
# baseline (speedup 1.0000x reference)
"""DLRM-ResNet (embedding_lookup) Trainium2 Bass kernel.

Strategy: data parallelism over the batch across 8 NeuronCores; each core
holds a full bf16 replica of the 2M x 128 table and processes 4096 rows.

The embedding gather is restructured as a 3-pass sort pipeline so the bulk
of the rows move through wide SWDGE dma_gather instructions (994ns fixed
overhead amortized over thousands of descriptors) instead of 832 small
indirect DMAs:

  pass A  (64 instrs): for each 32K-row table segment, gather that
          segment's rows for the whole core batch (host pre-buckets
          indices by (segment, batch-tile), int16 local ids, buckets
          padded to S=256 slots with dummy index 0) into an SBUF bounce.
  copy    (64 instrs): strided HWDGE copy bounce -> HBM staging laid out
          window-major: window t (batch tile of 512) owns a contiguous
          16512-row region; bucket (w,t) slot c=j*128+p lands at row
          w*256 + p*2 + j (copy-stream order), fixup tail at 16384.
  fixup   (1 indirect DMA): bucket-overflow pairs (rare) gathered with
          full-reach int32 indices, replicated into each window's tail.
  pass B  (8 instrs): per window, one transpose-mode dma_gather pulls the
          window's 26*512 rows from staging (int16 positions, host
          computed) directly into feature-major ze [128d, 26k*512b] bf16.

MLP per batch tile of 512 (feature-major, batch on free dim), as before:
  bottom MLP on host-pre-transposed dense f32 (f32r matmuls), top MLP
  layer 0 accumulates f32r h-part + 26 bf16 ze chunks in PSUM, relu+bias
  on ACT, residuals on DVE. Output [4096,1] f32, concatenated on host.
"""

import numpy as np
import ml_dtypes

import concourse.bass as bass
import concourse.bacc as bacc
import concourse.mybir as mybir
import concourse.tile as tile
from concourse.bass_utils import run_bass_kernel_spmd

F32 = mybir.dt.float32
F32R = mybir.dt.float32r
BF16 = mybir.dt.bfloat16
I16 = mybir.dt.int16
I32 = mybir.dt.int32

VOCAB = 2097152
D = 128          # embedding dim
NS = 26          # sparse features
ND = 13          # dense features
BATCH = 32768
NCORES = 8
P = 128

SEG = 32768              # table rows per segment (int16 reach)
NSEG = VOCAB // SEG      # 64
TB = 512                 # batch tile / window size
S = 256                  # bucket slots per (segment, window)
SJ = S // P              # 2
FIX = 128                # fixup slots per window
WROWS = NSEG * S + FIX   # 16512 staging rows per window

AF = mybir.ActivationFunctionType
ALU = mybir.AluOpType


def build_nc(bc: int):
    """Per-core program for a batch slice of `bc` rows (bc % TB == 0)."""
    nt = bc // TB            # windows / batch tiles
    npair = nt * S           # pass A num_idxs per segment

    nc = bacc.Bacc(
        "TRN2", target_bir_lowering=False, debug=False, num_devices=NCORES,
        dynamic_dma_scratch_size=32768,
    )

    xdT = nc.dram_tensor("xdT", [ND, bc], F32R, kind="ExternalInput")
    tab = nc.dram_tensor("tab", [VOCAB, D], BF16, kind="ExternalInput")
    idxa = nc.dram_tensor("idxa", [P, NSEG, npair // 16], I16, kind="ExternalInput")
    idxf = nc.dram_tensor("idxf", [P, 2], I32, kind="ExternalInput")
    idxb = nc.dram_tensor("idxb", [P, nt, (NS * TB) // 16], I16, kind="ExternalInput")
    wb0 = nc.dram_tensor("wb0", [ND, 256], F32R, kind="ExternalInput")
    wb12 = nc.dram_tensor("wb12", [P, 2, 2, 256], F32R, kind="ExternalInput")
    bbias = nc.dram_tensor("bbias", [P, 3, 2, 1], F32, kind="ExternalInput")
    w0h = nc.dram_tensor("w0h", [P, 2, 256], F32R, kind="ExternalInput")
    w0e = nc.dram_tensor("w0e", [P, NS, 256], BF16, kind="ExternalInput")
    wt123 = nc.dram_tensor("wt123", [P, 3, 2, 256], F32R, kind="ExternalInput")
    tbias = nc.dram_tensor("tbias", [P, 4, 2, 1], F32, kind="ExternalInput")
    w4 = nc.dram_tensor("w4", [P, 2, 1], F32R, kind="ExternalInput")
    tb4 = nc.dram_tensor("tb4", [1, 1], F32, kind="ExternalInput")
    out = nc.dram_tensor("out", [bc, 1], F32, kind="ExternalOutput")

    with tile.TileContext(nc) as tc:
        with (
            tc.tile_pool(name="stg", space="DRAM", bufs=1) as stgp,
            tc.tile_pool(name="wp", bufs=1) as wp,
            tc.tile_pool(name="bn", bufs=3) as bnp,
            tc.tile_pool(name="ze", bufs=2) as zep,
            tc.tile_pool(name="io", bufs=2) as io,
            tc.tile_pool(name="act", bufs=1) as actp,
            tc.tile_pool(name="psm", bufs=3, space="PSUM") as psm_pool,
            tc.tile_pool(name="pso", bufs=2, space="PSUM") as pso_pool,
        ):
            staging = stgp.tile([nt, WROWS, D], BF16)

            # ---- weight + index loads ----
            wb0_t = wp.tile([ND, 256], F32R)
            nc.sync.dma_start(wb0_t[:], wb0[:])
            wb12_t = wp.tile([P, 2, 2, 256], F32R)
            nc.sync.dma_start(wb12_t[:], wb12[:])
            bb_t = wp.tile([P, 3, 2, 1], F32)
            nc.sync.dma_start(bb_t[:], bbias[:])
            w0h_t = wp.tile([P, 2, 256], F32R)
            nc.sync.dma_start(w0h_t[:], w0h[:])
            w0e_t = wp.tile([P, NS, 256], BF16)
            nc.sync.dma_start(w0e_t[:], w0e[:])
            wt123_t = wp.tile([P, 3, 2, 256], F32R)
            nc.sync.dma_start(wt123_t[:], wt123[:])
            tb_t = wp.tile([P, 4, 2, 1], F32)
            nc.sync.dma_start(tb_t[:], tbias[:])
            w4_t = wp.tile([P, 2, 1], F32R)
            nc.sync.dma_start(w4_t[:], w4[:])
            tb4_t = wp.tile([1, 1], F32)
            nc.sync.dma_start(tb4_t[:], tb4[:])

            ia = wp.tile([P, NSEG, npair // 16], I16)
            nc.sync.dma_start(ia[:], idxa[:])
            ib = wp.tile([P, nt, (NS * TB) // 16], I16)
            nc.sync.dma_start(ib[:], idxb[:])
            ift = wp.tile([P, 2], I32)
            nc.sync.dma_start(ift[:], idxf[:])

            # ---- fixup first: full-reach indirect gather of overflow rows ----
            fb = wp.tile([P, 2, D], BF16)
            nc.gpsimd.indirect_dma_start(
                out=fb[:, 0, :],
                out_offset=None,
                in_=tab[:],
                in_offset=bass.IndirectOffsetOnAxis(ap=ift[:, 0:1], axis=0),
            )
            for t in range(nt):
                nc.sync.dma_start(
                    staging[t, NSEG * S : NSEG * S + FIX, :], fb[:, 0, :]
                )

            # ---- pass A: segmented gathers -> bounce -> staging ----
            for w in range(NSEG):
                bounce = bnp.tile([P, nt * SJ, D], BF16, tag=f"bn{w % 3}")
                nc.gpsimd.dma_gather(
                    out_ap=bounce[:],
                    in_ap=tab[w * SEG : (w + 1) * SEG, :],
                    idxs_ap=ia[:, w],
                    num_idxs=npair,
                    num_idxs_reg=npair,
                    elem_size=D,
                    single_packet=False,
                )
                src = bounce[:].rearrange("p (t j) e -> p t (j e)", t=nt, j=SJ)
                dst = staging[:, w * S : (w + 1) * S, :]
                dst = dst.rearrange("t (p j) e -> p t (j e)", p=P, j=SJ)
                nc.sync.dma_start(dst, src)

            # ---- per batch tile: pass B + MLP ----
            for t in range(nt):
                c0 = t * TB

                ze = zep.tile([P, 1, NS * TB], BF16, tag="ze")
                nc.gpsimd.dma_gather(
                    out_ap=ze[:],
                    in_ap=staging[t],
                    idxs_ap=ib[:, t],
                    num_idxs=NS * TB,
                    num_idxs_reg=NS * TB,
                    elem_size=D,
                    transpose=True,
                    single_packet=False,
                )

                dT = io.tile([ND, TB], F32R, tag="dT")
                nc.sync.dma_start(dT[:], xdT[:, c0 : c0 + TB])

                # ---- bottom MLP (feature-major) ----
                h1 = actp.tile([P, 2, TB], F32, tag="hA")
                h1r = actp.tile([P, 2, TB], F32R, tag="hrA")
                for m in range(2):
                    ps = psm_pool.tile([P, TB], F32, tag="psm")
                    nc.tensor.matmul(
                        ps[:],
                        wb0_t[:, m * P : (m + 1) * P],
                        dT[:],
                        start=True,
                        stop=True,
                    )
                    nc.scalar.activation(
                        h1[:, m, :], ps[:], AF.Relu, bias=bb_t[:, 0, m, :]
                    )
                    nc.vector.tensor_copy(h1r[:, m, :], h1[:, m, :])
                hprev, hprevr = h1, h1r
                for l in range(2):
                    hn = actp.tile([P, 2, TB], F32, tag=f"h{'BA'[l]}")
                    hnr = actp.tile([P, 2, TB], F32R, tag=f"hr{'BA'[l]}")
                    for m in range(2):
                        ps = psm_pool.tile([P, TB], F32, tag="psm")
                        for k in range(2):
                            nc.tensor.matmul(
                                ps[:],
                                wb12_t[:, l, k, m * P : (m + 1) * P],
                                hprevr[:, k, :],
                                start=(k == 0),
                                stop=(k == 1),
                            )
                        nc.scalar.activation(
                            hn[:, m, :], ps[:], AF.Relu, bias=bb_t[:, l + 1, m, :]
                        )
                        nc.vector.tensor_tensor(
                            hn[:, m, :], hn[:, m, :], hprev[:, m, :], op=ALU.add
                        )
                        nc.vector.tensor_copy(hnr[:, m, :], hn[:, m, :])
                    hprev, hprevr = hn, hnr

                # ---- top MLP layer 0: h-part (f32r) + 26 bf16 ze chunks ----
                z1 = actp.tile([P, 2, TB], F32, tag="zA")
                z1r = actp.tile([P, 2, TB], F32R, tag="zrA")
                for m in range(2):
                    ps = psm_pool.tile([P, TB], F32, tag="psm")
                    for k in range(2):
                        nc.tensor.matmul(
                            ps[:],
                            w0h_t[:, k, m * P : (m + 1) * P],
                            hprevr[:, k, :],
                            start=(k == 0),
                            stop=False,
                        )
                    for k in range(NS):
                        nc.tensor.matmul(
                            ps[:],
                            w0e_t[:, k, m * P : (m + 1) * P],
                            ze[:, 0, k * TB : (k + 1) * TB],
                            start=False,
                            stop=(k == NS - 1),
                        )
                    nc.scalar.activation(
                        z1[:, m, :], ps[:], AF.Relu, bias=tb_t[:, 0, m, :]
                    )
                    nc.vector.tensor_copy(z1r[:, m, :], z1[:, m, :])

                # ---- top residual layers 1..3 ----
                zprev, zprevr = z1, z1r
                for l in range(3):
                    zn = actp.tile([P, 2, TB], F32, tag=f"z{'BAB'[l]}")
                    znr = actp.tile([P, 2, TB], F32R, tag=f"zr{'BAB'[l]}")
                    for m in range(2):
                        ps = psm_pool.tile([P, TB], F32, tag="psm")
                        for k in range(2):
                            nc.tensor.matmul(
                                ps[:],
                                wt123_t[:, l, k, m * P : (m + 1) * P],
                                zprevr[:, k, :],
                                start=(k == 0),
                                stop=(k == 1),
                            )
                        nc.scalar.activation(
                            zn[:, m, :], ps[:], AF.Relu, bias=tb_t[:, l + 1, m, :]
                        )
                        nc.vector.tensor_tensor(
                            zn[:, m, :], zn[:, m, :], zprev[:, m, :], op=ALU.add
                        )
                        nc.vector.tensor_copy(znr[:, m, :], zn[:, m, :])
                    zprev, zprevr = zn, znr

                # ---- final linear [256 -> 1] ----
                po = pso_pool.tile([1, TB], F32, tag="pso")
                for k in range(2):
                    nc.tensor.matmul(
                        po[:],
                        w4_t[:, k, :],
                        zprevr[:, k, :],
                        start=(k == 0),
                        stop=(k == 1),
                    )
                ot = io.tile([1, TB], F32, tag="ot")
                nc.scalar.activation(
                    ot[:, :], po[:], AF.Identity, bias=tb4_t[:]
                )
                nc.sync.dma_start(out[c0 : c0 + TB, :], ot[:])

    nc.compile()
    return nc


def prep_weights(inp: dict) -> dict:
    """Host-side layout prep shared by all cores (all partition-major)."""
    f32 = np.float32
    bw0, bw1, bw2 = inp["bw0"], inp["bw1"], inp["bw2"]
    tw = [inp[f"tw{i}"] for i in range(5)]

    wb12 = np.stack(
        [w.T.reshape(2, P, 256).transpose(1, 0, 2) for w in (bw1, bw2)], axis=1
    )  # [128, 2(layer), 2(k), 256]
    bbias = np.stack(
        [inp[f"bb{i}"].reshape(2, P).T for i in range(3)], axis=1
    ).reshape(P, 3, 2, 1)

    t0T = tw[0].T  # [3584, 256]
    w0h = t0T[:256].reshape(2, P, 256).transpose(1, 0, 2)  # [128, 2, 256]
    w0e = (
        t0T[256:]
        .reshape(NS, P, 256)
        .transpose(1, 0, 2)
        .astype(ml_dtypes.bfloat16)
    )  # [128, 26, 256]
    wt123 = np.stack(
        [w.T.reshape(2, P, 256).transpose(1, 0, 2) for w in tw[1:4]], axis=1
    )  # [128, 3(layer), 2(k), 256]
    tbias = np.stack(
        [inp[f"tb{i}"].reshape(2, P).T for i in range(4)], axis=1
    ).reshape(P, 4, 2, 1)
    w4 = tw[4].T.reshape(2, P, 1).transpose(1, 0, 2)  # [128, 2, 1]
    tb4 = inp["tb4"].reshape(1, 1)

    tab = np.concatenate(
        [inp["emb0"], inp["emb1"], inp["emb2"], inp["emb3"]], axis=0
    ).astype(ml_dtypes.bfloat16)

    return {
        "wb0": np.ascontiguousarray(bw0.T, dtype=f32),
        "wb12": np.ascontiguousarray(wb12, dtype=f32),
        "bbias": np.ascontiguousarray(bbias, dtype=f32),
        "w0h": np.ascontiguousarray(w0h, dtype=f32),
        "w0e": np.ascontiguousarray(w0e),
        "wt123": np.ascontiguousarray(wt123.reshape(P, 3, 2, 256), dtype=f32),
        "tbias": np.ascontiguousarray(tbias, dtype=f32),
        "w4": np.ascontiguousarray(w4, dtype=f32),
        "tb4": np.ascontiguousarray(tb4, dtype=f32),
        "tab": np.ascontiguousarray(tab),
    }


def _pack16(lists: np.ndarray) -> np.ndarray:
    """[n_lists, n] int -> [128, n_lists, n//16] wrap-16 int16, 8x replicated."""
    nl, n = lists.shape
    w = lists.reshape(nl, n // 16, 16).transpose(0, 2, 1)  # [nl, 16, n//16]
    w = np.broadcast_to(w[:, None], (nl, 8, 16, n // 16)).reshape(nl, P, n // 16)
    return np.ascontiguousarray(w.transpose(1, 0, 2).astype(np.int16))


def _bucket_core(sidx: np.ndarray, nt: int):
    """Bucket one core's [bc, NS] int64 indices.

    Returns idxa [128, NSEG, npair//16] i16, idxf [128, 2] i32,
    idxb [128, nt, NS*TB//16] i16.
    """
    bc = sidx.shape[0]
    npair = nt * S
    b_of = np.repeat(np.arange(bc, dtype=np.int64), NS)
    k_of = np.tile(np.arange(NS, dtype=np.int64), bc)
    g = sidx.reshape(-1)                      # [bc*NS] global row
    t_of = b_of // TB
    w_of = g >> 15
    loc = g & 32767
    key = w_of * nt + t_of                    # bucket id, 0..NSEG*nt-1
    order = np.argsort(key, kind="stable")
    key_s = key[order]
    counts = np.bincount(key_s, minlength=NSEG * nt)
    starts = np.concatenate([[0], np.cumsum(counts)[:-1]])
    c = np.arange(bc * NS, dtype=np.int64) - starts[key_s]  # slot within bucket

    ok = c < S
    idxa = np.zeros((NSEG, npair), dtype=np.int64)
    w_s, t_s = w_of[order], t_of[order]
    idxa[w_s[ok], t_s[ok] * S + c[ok]] = loc[order][ok]
    # staging position: bucket (w,t) slot c=j*128+p -> row w*S + p*SJ + j
    pos_ok = w_s[ok] * S + (c[ok] % P) * SJ + c[ok] // P

    pos = np.zeros(bc * NS, dtype=np.int64)
    pos[order[ok]] = pos_ok
    ov = order[~ok]                            # overflow pair ids, sorted order
    nov = ov.shape[0]
    assert nov <= FIX, f"fixup overflow: {nov} > {FIX}"
    idxf = np.zeros((P,), dtype=np.int64)
    idxf[:nov] = g[ov]
    pos[ov] = NSEG * S + np.arange(nov)

    # pass B lists: column (k*TB + b%TB) of window t reads pos(b, k)
    idxb = np.zeros((nt, NS * TB), dtype=np.int64)
    idxb[t_of, k_of * TB + b_of % TB] = pos

    idxf2 = np.zeros((P, 2), dtype=np.int32)
    idxf2[:, 0] = idxf
    return _pack16(idxa), idxf2, _pack16(idxb)


def make_core_inputs(inp: dict, bc: int) -> list[dict]:
    """Shard x across cores; weights/table replicated; host-built indices."""
    shared = prep_weights(inp)
    x = np.asarray(inp["x"])
    nt = bc // TB
    in_maps = []
    for cidx in range(NCORES):
        xs = x[cidx * bc : (cidx + 1) * bc]
        sidx = xs[:, ND:].astype(np.int64) % VOCAB
        idxa, idxf, idxb = _bucket_core(sidx, nt)
        m = dict(shared)
        m["xdT"] = np.ascontiguousarray(xs[:, :ND].T, dtype=np.float32)
        m["idxa"] = idxa
        m["idxf"] = idxf
        m["idxb"] = idxb
        in_maps.append(m)
    return in_maps


_CACHE: dict = {}


def kernel(**inputs) -> np.ndarray:
    bc = BATCH // NCORES
    if "nc" not in _CACHE:
        _CACHE["nc"] = build_nc(bc)
    nc = _CACHE["nc"]
    in_maps = make_core_inputs(inputs, bc)
    res = run_bass_kernel_spmd(nc, in_maps, core_ids=list(range(NCORES)))
    outs = [res.results[c]["out"] for c in range(NCORES)]
    return np.concatenate(outs, axis=0).astype(np.float32)


# revision 7
# speedup vs baseline: 1.0518x; 1.0518x over previous
"""DLRM-ResNet (embedding_lookup) Trainium2 Bass kernel.

Strategy: data parallelism over the batch across 8 NeuronCores; each core
holds a full bf16 replica of the 2M x 128 table and processes 4096 rows.

The embedding gather is restructured as a 3-pass sort pipeline so the bulk
of the rows move through wide SWDGE dma_gather instructions (994ns fixed
overhead amortized over thousands of descriptors) instead of 832 small
indirect DMAs:

  pass A  (64 instrs): for each 32K-row table segment, gather that
          segment's rows for the whole core batch (host pre-buckets
          indices by (segment, batch-tile), int16 local ids, buckets
          padded to S=256 slots with dummy index 0) into an SBUF bounce.
  copy    (64 instrs): strided HWDGE copy bounce -> HBM staging laid out
          window-major: window t (batch tile of 512) owns a contiguous
          16512-row region; bucket (w,t) slot c=j*128+p lands at row
          w*256 + p*2 + j (copy-stream order), fixup tail at 16384.
  fixup   (1 indirect DMA): bucket-overflow pairs (rare) gathered with
          full-reach int32 indices, replicated into each window's tail.
  pass B  (8 instrs): per window, one transpose-mode dma_gather pulls the
          window's 26*512 rows from staging (int16 positions, host
          computed) directly into feature-major ze [128d, 26k*512b] bf16.

MLP per batch tile of 512 (feature-major, batch on free dim), as before:
  bottom MLP on host-pre-transposed dense f32 (f32r matmuls), top MLP
  layer 0 accumulates f32r h-part + 26 bf16 ze chunks in PSUM, relu+bias
  on ACT, residuals on DVE. Output [4096,1] f32, concatenated on host.
"""

import numpy as np
import ml_dtypes

import concourse.bass as bass
import concourse.bacc as bacc
import concourse.mybir as mybir
import concourse.tile as tile
from concourse.bass_utils import run_bass_kernel_spmd

F32 = mybir.dt.float32
F32R = mybir.dt.float32r
BF16 = mybir.dt.bfloat16
I16 = mybir.dt.int16
I32 = mybir.dt.int32

VOCAB = 2097152
D = 128          # embedding dim
NS = 26          # sparse features
ND = 13          # dense features
BATCH = 32768
NCORES = 8
P = 128

SEG = 32768              # table rows per segment (int16 reach)
NSEG = VOCAB // SEG      # 64
TB = 512                 # batch tile / window size
S = 256                  # bucket slots per (segment, window)
SJ = S // P              # 2
FIX = 128                # fixup slots per window
WROWS = NSEG * S + FIX   # 16512 staging rows per window

AF = mybir.ActivationFunctionType
ALU = mybir.AluOpType


def build_nc(bc: int):
    """Per-core program for a batch slice of `bc` rows (bc % TB == 0)."""
    nt = bc // TB            # windows / batch tiles
    npair = nt * S           # pass A num_idxs per segment

    nc = bacc.Bacc(
        "TRN2", target_bir_lowering=False, debug=False, num_devices=NCORES
    )

    xdT = nc.dram_tensor("xdT", [ND, bc], F32R, kind="ExternalInput")
    tab = nc.dram_tensor("tab", [VOCAB, D], BF16, kind="ExternalInput")
    idxa = nc.dram_tensor("idxa", [P, NSEG, npair // 16], I16, kind="ExternalInput")
    idxf = nc.dram_tensor("idxf", [P, 2], I32, kind="ExternalInput")
    idxb = nc.dram_tensor("idxb", [P, nt, (NS * TB) // 16], I16, kind="ExternalInput")
    wb0 = nc.dram_tensor("wb0", [ND, 256], F32R, kind="ExternalInput")
    wb12 = nc.dram_tensor("wb12", [P, 2, 2, 256], F32R, kind="ExternalInput")
    bbias = nc.dram_tensor("bbias", [P, 3, 2, 1], F32, kind="ExternalInput")
    w0h = nc.dram_tensor("w0h", [P, 2, 256], F32R, kind="ExternalInput")
    w0e = nc.dram_tensor("w0e", [P, NS, 256], BF16, kind="ExternalInput")
    wt123 = nc.dram_tensor("wt123", [P, 3, 2, 256], F32R, kind="ExternalInput")
    tbias = nc.dram_tensor("tbias", [P, 4, 2, 1], F32, kind="ExternalInput")
    w4 = nc.dram_tensor("w4", [P, 2, 1], F32R, kind="ExternalInput")
    tb4 = nc.dram_tensor("tb4", [1, 1], F32, kind="ExternalInput")
    out = nc.dram_tensor("out", [bc, 1], F32, kind="ExternalOutput")

    with tile.TileContext(nc) as tc:
        with (
            tc.tile_pool(name="stg", space="DRAM", bufs=1) as stgp,
            tc.tile_pool(name="wp", bufs=1) as wp,
            tc.tile_pool(name="bn", bufs=3) as bnp,
            tc.tile_pool(name="ze", bufs=3) as zep,
            tc.tile_pool(name="io", bufs=2) as io,
            tc.tile_pool(name="act", bufs=2) as actp,
            tc.tile_pool(name="psm", bufs=3, space="PSUM") as psm_pool,
            tc.tile_pool(name="pso", bufs=2, space="PSUM") as pso_pool,
        ):
            staging = stgp.tile([nt, WROWS, D], BF16)

            # ---- weight + index loads ----
            wb0_t = wp.tile([ND, 256], F32R)
            nc.sync.dma_start(wb0_t[:], wb0[:])
            wb12_t = wp.tile([P, 2, 2, 256], F32R)
            nc.sync.dma_start(wb12_t[:], wb12[:])
            bb_t = wp.tile([P, 3, 2, 1], F32)
            nc.sync.dma_start(bb_t[:], bbias[:])
            w0h_t = wp.tile([P, 2, 256], F32R)
            nc.sync.dma_start(w0h_t[:], w0h[:])
            w0e_t = wp.tile([P, NS, 256], BF16)
            nc.sync.dma_start(w0e_t[:], w0e[:])
            wt123_t = wp.tile([P, 3, 2, 256], F32R)
            nc.sync.dma_start(wt123_t[:], wt123[:])
            tb_t = wp.tile([P, 4, 2, 1], F32)
            nc.sync.dma_start(tb_t[:], tbias[:])
            w4_t = wp.tile([P, 2, 1], F32R)
            nc.sync.dma_start(w4_t[:], w4[:])
            tb4_t = wp.tile([1, 1], F32)
            nc.sync.dma_start(tb4_t[:], tb4[:])

            ib = wp.tile([P, nt, (NS * TB) // 16], I16)
            nc.sync.dma_start(ib[:], idxb[:])
            ift = wp.tile([P, 2], I32)
            nc.sync.dma_start(ift[:], idxf[:])

            # ---- fixup first: full-reach indirect gather of overflow rows ----
            fb = wp.tile([P, 2, D], BF16)
            nc.gpsimd.indirect_dma_start(
                out=fb[:, 0, :],
                out_offset=None,
                in_=tab[:],
                in_offset=bass.IndirectOffsetOnAxis(ap=ift[:, 0:1], axis=0),
            )
            for t in range(nt):
                nc.sync.dma_start(
                    staging[t, NSEG * S : NSEG * S + FIX, :], fb[:, 0, :]
                )

            # ---- pass A: segmented gathers -> bounce -> staging ----
            IAC = 8  # segments per index-chunk load
            for w in range(NSEG):
                if w % IAC == 0:
                    ia = io.tile([P, IAC, npair // 16], I16, tag=f"ia{(w // IAC) % 2}")
                    nc.sync.dma_start(ia[:], idxa[:, w : w + IAC])
                bounce = bnp.tile([P, nt * SJ, D], BF16, tag=f"bn{w % 3}")
                nc.gpsimd.dma_gather(
                    out_ap=bounce[:],
                    in_ap=tab[w * SEG : (w + 1) * SEG, :],
                    idxs_ap=ia[:, w % IAC],
                    num_idxs=npair,
                    num_idxs_reg=npair,
                    elem_size=D,
                    single_packet=False,
                )
                src = bounce[:].rearrange("p (t j) e -> p t (j e)", t=nt, j=SJ)
                dst = staging[:, w * S : (w + 1) * S, :]
                dst = dst.rearrange("t (p j) e -> p t (j e)", p=P, j=SJ)
                nc.sync.dma_start(dst, src)

            # ---- per batch tile: pass B + MLP ----
            for t in range(nt):
                c0 = t * TB

                ze = zep.tile([P, 1, NS * TB], BF16, tag="ze")
                nc.gpsimd.dma_gather(
                    out_ap=ze[:],
                    in_ap=staging[t],
                    idxs_ap=ib[:, t],
                    num_idxs=NS * TB,
                    num_idxs_reg=NS * TB,
                    elem_size=D,
                    transpose=True,
                    single_packet=False,
                )

                dT = io.tile([ND, TB], F32R, tag="dT")
                nc.sync.dma_start(dT[:], xdT[:, c0 : c0 + TB])

                # ---- bottom MLP (feature-major) ----
                h1 = actp.tile([P, 2, TB], F32, tag="hA")
                for m in range(2):
                    ps = psm_pool.tile([P, TB], F32, tag="psm")
                    nc.tensor.matmul(
                        ps[:],
                        wb0_t[:, m * P : (m + 1) * P],
                        dT[:],
                        start=True,
                        stop=True,
                    )
                    nc.scalar.activation(
                        h1[:, m, :], ps[:], AF.Relu, bias=bb_t[:, 0, m, :]
                    )
                hprev = h1
                for l in range(2):
                    hn = actp.tile([P, 2, TB], F32, tag=f"h{'BA'[l]}")
                    for m in range(2):
                        ps = psm_pool.tile([P, TB], F32, tag="psm")
                        for k in range(2):
                            nc.tensor.matmul(
                                ps[:],
                                wb12_t[:, l, k, m * P : (m + 1) * P],
                                hprev[:, k, :].bitcast(F32R),
                                start=(k == 0),
                                stop=(k == 1),
                            )
                        nc.scalar.activation(
                            hn[:, m, :], ps[:], AF.Relu, bias=bb_t[:, l + 1, m, :]
                        )
                        nc.vector.tensor_tensor(
                            hn[:, m, :], hn[:, m, :], hprev[:, m, :], op=ALU.add
                        )
                    hprev = hn

                # ---- top MLP layer 0: h-part (f32r) + 26 bf16 ze chunks ----
                z1 = actp.tile([P, 2, TB], F32, tag="zA")
                for m in range(2):
                    ps = psm_pool.tile([P, TB], F32, tag="psm")
                    for k in range(2):
                        nc.tensor.matmul(
                            ps[:],
                            w0h_t[:, k, m * P : (m + 1) * P],
                            hprev[:, k, :].bitcast(F32R),
                            start=(k == 0),
                            stop=False,
                        )
                    for k in range(NS):
                        nc.tensor.matmul(
                            ps[:],
                            w0e_t[:, k, m * P : (m + 1) * P],
                            ze[:, 0, k * TB : (k + 1) * TB],
                            start=False,
                            stop=(k == NS - 1),
                        )
                    nc.scalar.activation(
                        z1[:, m, :], ps[:], AF.Relu, bias=tb_t[:, 0, m, :]
                    )

                # ---- top residual layers 1..3 ----
                zprev = z1
                for l in range(3):
                    zn = actp.tile([P, 2, TB], F32, tag=f"z{'BAB'[l]}")
                    for m in range(2):
                        ps = psm_pool.tile([P, TB], F32, tag="psm")
                        for k in range(2):
                            nc.tensor.matmul(
                                ps[:],
                                wt123_t[:, l, k, m * P : (m + 1) * P],
                                zprev[:, k, :].bitcast(F32R),
                                start=(k == 0),
                                stop=(k == 1),
                            )
                        nc.scalar.activation(
                            zn[:, m, :], ps[:], AF.Relu, bias=tb_t[:, l + 1, m, :]
                        )
                        nc.vector.tensor_tensor(
                            zn[:, m, :], zn[:, m, :], zprev[:, m, :], op=ALU.add
                        )
                    zprev = zn

                # ---- final linear [256 -> 1] ----
                po = pso_pool.tile([1, TB], F32, tag="pso")
                for k in range(2):
                    nc.tensor.matmul(
                        po[:],
                        w4_t[:, k, :],
                        zprev[:, k, :].bitcast(F32R),
                        start=(k == 0),
                        stop=(k == 1),
                    )
                ot = io.tile([1, TB], F32, tag="ot")
                nc.scalar.activation(
                    ot[:, :], po[:], AF.Identity, bias=tb4_t[:]
                )
                nc.sync.dma_start(out[c0 : c0 + TB, :], ot[:])

    nc.compile()
    return nc


def prep_weights(inp: dict) -> dict:
    """Host-side layout prep shared by all cores (all partition-major)."""
    f32 = np.float32
    bw0, bw1, bw2 = inp["bw0"], inp["bw1"], inp["bw2"]
    tw = [inp[f"tw{i}"] for i in range(5)]

    wb12 = np.stack(
        [w.T.reshape(2, P, 256).transpose(1, 0, 2) for w in (bw1, bw2)], axis=1
    )  # [128, 2(layer), 2(k), 256]
    bbias = np.stack(
        [inp[f"bb{i}"].reshape(2, P).T for i in range(3)], axis=1
    ).reshape(P, 3, 2, 1)

    t0T = tw[0].T  # [3584, 256]
    w0h = t0T[:256].reshape(2, P, 256).transpose(1, 0, 2)  # [128, 2, 256]
    w0e = (
        t0T[256:]
        .reshape(NS, P, 256)
        .transpose(1, 0, 2)
        .astype(ml_dtypes.bfloat16)
    )  # [128, 26, 256]
    wt123 = np.stack(
        [w.T.reshape(2, P, 256).transpose(1, 0, 2) for w in tw[1:4]], axis=1
    )  # [128, 3(layer), 2(k), 256]
    tbias = np.stack(
        [inp[f"tb{i}"].reshape(2, P).T for i in range(4)], axis=1
    ).reshape(P, 4, 2, 1)
    w4 = tw[4].T.reshape(2, P, 1).transpose(1, 0, 2)  # [128, 2, 1]
    tb4 = inp["tb4"].reshape(1, 1)

    tab = np.concatenate(
        [inp["emb0"], inp["emb1"], inp["emb2"], inp["emb3"]], axis=0
    ).astype(ml_dtypes.bfloat16)

    return {
        "wb0": np.ascontiguousarray(bw0.T, dtype=f32),
        "wb12": np.ascontiguousarray(wb12, dtype=f32),
        "bbias": np.ascontiguousarray(bbias, dtype=f32),
        "w0h": np.ascontiguousarray(w0h, dtype=f32),
        "w0e": np.ascontiguousarray(w0e),
        "wt123": np.ascontiguousarray(wt123.reshape(P, 3, 2, 256), dtype=f32),
        "tbias": np.ascontiguousarray(tbias, dtype=f32),
        "w4": np.ascontiguousarray(w4, dtype=f32),
        "tb4": np.ascontiguousarray(tb4, dtype=f32),
        "tab": np.ascontiguousarray(tab),
    }


def _pack16(lists: np.ndarray) -> np.ndarray:
    """[n_lists, n] int -> [128, n_lists, n//16] wrap-16 int16, 8x replicated."""
    nl, n = lists.shape
    w = lists.reshape(nl, n // 16, 16).transpose(0, 2, 1)  # [nl, 16, n//16]
    w = np.broadcast_to(w[:, None], (nl, 8, 16, n // 16)).reshape(nl, P, n // 16)
    return np.ascontiguousarray(w.transpose(1, 0, 2).astype(np.int16))


def _bucket_core(sidx: np.ndarray, nt: int):
    """Bucket one core's [bc, NS] int64 indices.

    Returns idxa [128, NSEG, npair//16] i16, idxf [128, 2] i32,
    idxb [128, nt, NS*TB//16] i16.
    """
    bc = sidx.shape[0]
    npair = nt * S
    b_of = np.repeat(np.arange(bc, dtype=np.int64), NS)
    k_of = np.tile(np.arange(NS, dtype=np.int64), bc)
    g = sidx.reshape(-1)                      # [bc*NS] global row
    t_of = b_of // TB
    w_of = g >> 15
    loc = g & 32767
    key = w_of * nt + t_of                    # bucket id, 0..NSEG*nt-1
    order = np.argsort(key, kind="stable")
    key_s = key[order]
    counts = np.bincount(key_s, minlength=NSEG * nt)
    starts = np.concatenate([[0], np.cumsum(counts)[:-1]])
    c = np.arange(bc * NS, dtype=np.int64) - starts[key_s]  # slot within bucket

    ok = c < S
    idxa = np.zeros((NSEG, npair), dtype=np.int64)
    w_s, t_s = w_of[order], t_of[order]
    idxa[w_s[ok], t_s[ok] * S + c[ok]] = loc[order][ok]
    # staging position: bucket (w,t) slot c=j*128+p -> row w*S + p*SJ + j
    pos_ok = w_s[ok] * S + (c[ok] % P) * SJ + c[ok] // P

    pos = np.zeros(bc * NS, dtype=np.int64)
    pos[order[ok]] = pos_ok
    ov = order[~ok]                            # overflow pair ids, sorted order
    nov = ov.shape[0]
    assert nov <= FIX, f"fixup overflow: {nov} > {FIX}"
    idxf = np.zeros((P,), dtype=np.int64)
    idxf[:nov] = g[ov]
    pos[ov] = NSEG * S + np.arange(nov)

    # pass B lists: column (k*TB + b%TB) of window t reads pos(b, k)
    idxb = np.zeros((nt, NS * TB), dtype=np.int64)
    idxb[t_of, k_of * TB + b_of % TB] = pos

    idxf2 = np.zeros((P, 2), dtype=np.int32)
    idxf2[:, 0] = idxf
    return _pack16(idxa), idxf2, _pack16(idxb)


def make_core_inputs(inp: dict, bc: int) -> list[dict]:
    """Shard x across cores; weights/table replicated; host-built indices."""
    shared = prep_weights(inp)
    x = np.asarray(inp["x"])
    nt = bc // TB
    in_maps = []
    for cidx in range(NCORES):
        xs = x[cidx * bc : (cidx + 1) * bc]
        sidx = xs[:, ND:].astype(np.int64) % VOCAB
        idxa, idxf, idxb = _bucket_core(sidx, nt)
        m = dict(shared)
        m["xdT"] = np.ascontiguousarray(xs[:, :ND].T, dtype=np.float32)
        m["idxa"] = idxa
        m["idxf"] = idxf
        m["idxb"] = idxb
        in_maps.append(m)
    return in_maps


_CACHE: dict = {}


def kernel(**inputs) -> np.ndarray:
    bc = BATCH // NCORES
    if "nc" not in _CACHE:
        _CACHE["nc"] = build_nc(bc)
    nc = _CACHE["nc"]
    in_maps = make_core_inputs(inputs, bc)
    res = run_bass_kernel_spmd(nc, in_maps, core_ids=list(range(NCORES)))
    outs = [res.results[c]["out"] for c in range(NCORES)]
    return np.concatenate(outs, axis=0).astype(np.float32)


# revision 9
# speedup vs baseline: 1.0541x; 1.0022x over previous
"""DLRM-ResNet (embedding_lookup) Trainium2 Bass kernel.

Strategy: data parallelism over the batch across 8 NeuronCores; each core
holds a full bf16 replica of the 2M x 128 table and processes 4096 rows.

The embedding gather is restructured as a 3-pass sort pipeline so the bulk
of the rows move through wide SWDGE dma_gather instructions (994ns fixed
overhead amortized over thousands of descriptors) instead of 832 small
indirect DMAs:

  pass A  (64 instrs): for each 32K-row table segment, gather that
          segment's rows for the whole core batch (host pre-buckets
          indices by (segment, batch-tile), int16 local ids, buckets
          padded to S=256 slots with dummy index 0) into an SBUF bounce.
  copy    (64 instrs): strided HWDGE copy bounce -> HBM staging laid out
          window-major: window t (batch tile of 512) owns a contiguous
          16512-row region; bucket (w,t) slot c=j*128+p lands at row
          w*256 + p*2 + j (copy-stream order), fixup tail at 16384.
  fixup   (1 indirect DMA): bucket-overflow pairs (rare) gathered with
          full-reach int32 indices, replicated into each window's tail.
  pass B  (8 instrs): per window, one transpose-mode dma_gather pulls the
          window's 26*512 rows from staging (int16 positions, host
          computed) directly into feature-major ze [128d, 26k*512b] bf16.

MLP per batch tile of 512 (feature-major, batch on free dim), as before:
  bottom MLP on host-pre-transposed dense f32 (f32r matmuls), top MLP
  layer 0 accumulates f32r h-part + 26 bf16 ze chunks in PSUM, relu+bias
  on ACT, residuals on DVE. Output [4096,1] f32, concatenated on host.
"""

import numpy as np
import ml_dtypes

import concourse.bass as bass
import concourse.bacc as bacc
import concourse.mybir as mybir
import concourse.tile as tile
from concourse.bass_utils import run_bass_kernel_spmd

F32 = mybir.dt.float32
F32R = mybir.dt.float32r
BF16 = mybir.dt.bfloat16
I16 = mybir.dt.int16
I32 = mybir.dt.int32

VOCAB = 2097152
D = 128          # embedding dim
NS = 26          # sparse features
ND = 13          # dense features
BATCH = 32768
NCORES = 8
P = 128

SEG = 32768              # table rows per segment (int16 reach)
NSEG = VOCAB // SEG      # 64
TB = 512                 # batch tile / window size
S = 256                  # bucket slots per (segment, window)
SJ = S // P              # 2
FIX = 128                # fixup slots per window
WROWS = NSEG * S + FIX   # 16512 staging rows per window

AF = mybir.ActivationFunctionType
ALU = mybir.AluOpType


def build_nc(bc: int):
    """Per-core program for a batch slice of `bc` rows (bc % TB == 0)."""
    nt = bc // TB            # windows / batch tiles
    npair = nt * S           # pass A num_idxs per segment

    nc = bacc.Bacc(
        "TRN2", target_bir_lowering=False, debug=False, num_devices=NCORES,
        dynamic_dma_scratch_size=40960,
    )

    xdT = nc.dram_tensor("xdT", [ND, bc], F32R, kind="ExternalInput")
    tab = nc.dram_tensor("tab", [VOCAB, D], BF16, kind="ExternalInput")
    idxa = nc.dram_tensor("idxa", [P, NSEG, npair // 16], I16, kind="ExternalInput")
    idxf = nc.dram_tensor("idxf", [P, 2], I32, kind="ExternalInput")
    idxb = nc.dram_tensor("idxb", [P, nt, (NS * TB) // 16], I16, kind="ExternalInput")
    wb0 = nc.dram_tensor("wb0", [ND, 256], F32R, kind="ExternalInput")
    wb12 = nc.dram_tensor("wb12", [P, 2, 2, 256], F32R, kind="ExternalInput")
    bbias = nc.dram_tensor("bbias", [P, 3, 2, 1], F32, kind="ExternalInput")
    w0h = nc.dram_tensor("w0h", [P, 2, 256], F32R, kind="ExternalInput")
    w0e = nc.dram_tensor("w0e", [P, NS, 256], BF16, kind="ExternalInput")
    wt123 = nc.dram_tensor("wt123", [P, 3, 2, 256], F32R, kind="ExternalInput")
    tbias = nc.dram_tensor("tbias", [P, 4, 2, 1], F32, kind="ExternalInput")
    w4 = nc.dram_tensor("w4", [P, 2, 1], F32R, kind="ExternalInput")
    tb4 = nc.dram_tensor("tb4", [1, 1], F32, kind="ExternalInput")
    out = nc.dram_tensor("out", [bc, 1], F32, kind="ExternalOutput")

    with tile.TileContext(nc) as tc:
        with (
            tc.tile_pool(name="stg", space="DRAM", bufs=1) as stgp,
            tc.tile_pool(name="wp", bufs=1) as wp,
            tc.tile_pool(name="bn", bufs=2) as bnp,
            tc.tile_pool(name="ze", bufs=2) as zep,
            tc.tile_pool(name="io", bufs=2) as io,
            tc.tile_pool(name="act", bufs=2) as actp,
            tc.tile_pool(name="psm", bufs=3, space="PSUM") as psm_pool,
            tc.tile_pool(name="pso", bufs=2, space="PSUM") as pso_pool,
        ):
            staging = stgp.tile([nt, WROWS, D], BF16)

            # ---- weight + index loads ----
            wb0_t = wp.tile([ND, 256], F32R)
            nc.sync.dma_start(wb0_t[:], wb0[:])
            wb12_t = wp.tile([P, 2, 2, 256], F32R)
            nc.sync.dma_start(wb12_t[:], wb12[:])
            bb_t = wp.tile([P, 3, 2, 1], F32)
            nc.sync.dma_start(bb_t[:], bbias[:])
            w0h_t = wp.tile([P, 2, 256], F32R)
            nc.sync.dma_start(w0h_t[:], w0h[:])
            w0e_t = wp.tile([P, NS, 256], BF16)
            nc.sync.dma_start(w0e_t[:], w0e[:])
            wt123_t = wp.tile([P, 3, 2, 256], F32R)
            nc.sync.dma_start(wt123_t[:], wt123[:])
            tb_t = wp.tile([P, 4, 2, 1], F32)
            nc.sync.dma_start(tb_t[:], tbias[:])
            w4_t = wp.tile([P, 2, 1], F32R)
            nc.sync.dma_start(w4_t[:], w4[:])
            tb4_t = wp.tile([1, 1], F32)
            nc.sync.dma_start(tb4_t[:], tb4[:])

            ib = wp.tile([P, nt, (NS * TB) // 16], I16)
            nc.sync.dma_start(ib[:], idxb[:])
            ift = wp.tile([P, 2], I32)
            nc.sync.dma_start(ift[:], idxf[:])

            # ---- fixup first: full-reach indirect gather of overflow rows ----
            fb = wp.tile([P, 2, D], BF16)
            nc.gpsimd.indirect_dma_start(
                out=fb[:, 0, :],
                out_offset=None,
                in_=tab[:],
                in_offset=bass.IndirectOffsetOnAxis(ap=ift[:, 0:1], axis=0),
            )
            for t in range(nt):
                nc.sync.dma_start(
                    staging[t, NSEG * S : NSEG * S + FIX, :], fb[:, 0, :]
                )

            # ---- pass A: segmented gathers -> bounce -> staging ----
            IAC = 8  # segments per index-chunk load
            for w in range(NSEG):
                if w % IAC == 0:
                    ia = io.tile([P, IAC, npair // 16], I16, tag=f"ia{(w // IAC) % 2}")
                    nc.sync.dma_start(ia[:], idxa[:, w : w + IAC])
                bounce = bnp.tile([P, nt * SJ, D], BF16, tag=f"bn{w % 2}")
                nc.gpsimd.dma_gather(
                    out_ap=bounce[:],
                    in_ap=tab[w * SEG : (w + 1) * SEG, :],
                    idxs_ap=ia[:, w % IAC],
                    num_idxs=npair,
                    num_idxs_reg=npair,
                    elem_size=D,
                    single_packet=False,
                )
                src = bounce[:].rearrange("p (t j) e -> p t (j e)", t=nt, j=SJ)
                dst = staging[:, w * S : (w + 1) * S, :]
                dst = dst.rearrange("t (p j) e -> p t (j e)", p=P, j=SJ)
                nc.sync.dma_start(dst, src)

            # ---- per batch tile: pass B + MLP ----
            for t in range(nt):
                c0 = t * TB

                ze = zep.tile([P, 1, NS * TB], BF16, tag="ze")
                nc.gpsimd.dma_gather(
                    out_ap=ze[:],
                    in_ap=staging[t],
                    idxs_ap=ib[:, t],
                    num_idxs=NS * TB,
                    num_idxs_reg=NS * TB,
                    elem_size=D,
                    transpose=True,
                    single_packet=False,
                )

                dT = io.tile([ND, TB], F32R, tag="dT")
                nc.sync.dma_start(dT[:], xdT[:, c0 : c0 + TB])

                # ---- bottom MLP (feature-major) ----
                h1 = actp.tile([P, 2, TB], F32, tag="hA")
                for m in range(2):
                    ps = psm_pool.tile([P, TB], F32, tag="psm")
                    nc.tensor.matmul(
                        ps[:],
                        wb0_t[:, m * P : (m + 1) * P],
                        dT[:],
                        start=True,
                        stop=True,
                    )
                    nc.scalar.activation(
                        h1[:, m, :], ps[:], AF.Relu, bias=bb_t[:, 0, m, :]
                    )
                hprev = h1
                for l in range(2):
                    hn = actp.tile([P, 2, TB], F32, tag=f"h{'BA'[l]}")
                    for m in range(2):
                        ps = psm_pool.tile([P, TB], F32, tag="psm")
                        for k in range(2):
                            nc.tensor.matmul(
                                ps[:],
                                wb12_t[:, l, k, m * P : (m + 1) * P],
                                hprev[:, k, :].bitcast(F32R),
                                start=(k == 0),
                                stop=(k == 1),
                            )
                        nc.scalar.activation(
                            hn[:, m, :], ps[:], AF.Relu, bias=bb_t[:, l + 1, m, :]
                        )
                        nc.vector.tensor_tensor(
                            hn[:, m, :], hn[:, m, :], hprev[:, m, :], op=ALU.add
                        )
                    hprev = hn

                # ---- top MLP layer 0: h-part (f32r) + 26 bf16 ze chunks ----
                z1 = actp.tile([P, 2, TB], F32, tag="zA")
                for m in range(2):
                    ps = psm_pool.tile([P, TB], F32, tag="psm")
                    for k in range(2):
                        nc.tensor.matmul(
                            ps[:],
                            w0h_t[:, k, m * P : (m + 1) * P],
                            hprev[:, k, :].bitcast(F32R),
                            start=(k == 0),
                            stop=False,
                        )
                    for k in range(NS):
                        nc.tensor.matmul(
                            ps[:],
                            w0e_t[:, k, m * P : (m + 1) * P],
                            ze[:, 0, k * TB : (k + 1) * TB],
                            start=False,
                            stop=(k == NS - 1),
                        )
                    nc.scalar.activation(
                        z1[:, m, :], ps[:], AF.Relu, bias=tb_t[:, 0, m, :]
                    )

                # ---- top residual layers 1..3 ----
                zprev = z1
                for l in range(3):
                    zn = actp.tile([P, 2, TB], F32, tag=f"z{'BAB'[l]}")
                    for m in range(2):
                        ps = psm_pool.tile([P, TB], F32, tag="psm")
                        for k in range(2):
                            nc.tensor.matmul(
                                ps[:],
                                wt123_t[:, l, k, m * P : (m + 1) * P],
                                zprev[:, k, :].bitcast(F32R),
                                start=(k == 0),
                                stop=(k == 1),
                            )
                        nc.scalar.activation(
                            zn[:, m, :], ps[:], AF.Relu, bias=tb_t[:, l + 1, m, :]
                        )
                        nc.vector.tensor_tensor(
                            zn[:, m, :], zn[:, m, :], zprev[:, m, :], op=ALU.add
                        )
                    zprev = zn

                # ---- final linear [256 -> 1] ----
                po = pso_pool.tile([1, TB], F32, tag="pso")
                for k in range(2):
                    nc.tensor.matmul(
                        po[:],
                        w4_t[:, k, :],
                        zprev[:, k, :].bitcast(F32R),
                        start=(k == 0),
                        stop=(k == 1),
                    )
                ot = io.tile([1, TB], F32, tag="ot")
                nc.scalar.activation(
                    ot[:, :], po[:], AF.Identity, bias=tb4_t[:]
                )
                nc.sync.dma_start(out[c0 : c0 + TB, :], ot[:])

    nc.compile()
    return nc


def prep_weights(inp: dict) -> dict:
    """Host-side layout prep shared by all cores (all partition-major)."""
    f32 = np.float32
    bw0, bw1, bw2 = inp["bw0"], inp["bw1"], inp["bw2"]
    tw = [inp[f"tw{i}"] for i in range(5)]

    wb12 = np.stack(
        [w.T.reshape(2, P, 256).transpose(1, 0, 2) for w in (bw1, bw2)], axis=1
    )  # [128, 2(layer), 2(k), 256]
    bbias = np.stack(
        [inp[f"bb{i}"].reshape(2, P).T for i in range(3)], axis=1
    ).reshape(P, 3, 2, 1)

    t0T = tw[0].T  # [3584, 256]
    w0h = t0T[:256].reshape(2, P, 256).transpose(1, 0, 2)  # [128, 2, 256]
    w0e = (
        t0T[256:]
        .reshape(NS, P, 256)
        .transpose(1, 0, 2)
        .astype(ml_dtypes.bfloat16)
    )  # [128, 26, 256]
    wt123 = np.stack(
        [w.T.reshape(2, P, 256).transpose(1, 0, 2) for w in tw[1:4]], axis=1
    )  # [128, 3(layer), 2(k), 256]
    tbias = np.stack(
        [inp[f"tb{i}"].reshape(2, P).T for i in range(4)], axis=1
    ).reshape(P, 4, 2, 1)
    w4 = tw[4].T.reshape(2, P, 1).transpose(1, 0, 2)  # [128, 2, 1]
    tb4 = inp["tb4"].reshape(1, 1)

    tab = np.concatenate(
        [inp["emb0"], inp["emb1"], inp["emb2"], inp["emb3"]], axis=0
    ).astype(ml_dtypes.bfloat16)

    return {
        "wb0": np.ascontiguousarray(bw0.T, dtype=f32),
        "wb12": np.ascontiguousarray(wb12, dtype=f32),
        "bbias": np.ascontiguousarray(bbias, dtype=f32),
        "w0h": np.ascontiguousarray(w0h, dtype=f32),
        "w0e": np.ascontiguousarray(w0e),
        "wt123": np.ascontiguousarray(wt123.reshape(P, 3, 2, 256), dtype=f32),
        "tbias": np.ascontiguousarray(tbias, dtype=f32),
        "w4": np.ascontiguousarray(w4, dtype=f32),
        "tb4": np.ascontiguousarray(tb4, dtype=f32),
        "tab": np.ascontiguousarray(tab),
    }


def _pack16(lists: np.ndarray) -> np.ndarray:
    """[n_lists, n] int -> [128, n_lists, n//16] wrap-16 int16, 8x replicated."""
    nl, n = lists.shape
    w = lists.reshape(nl, n // 16, 16).transpose(0, 2, 1)  # [nl, 16, n//16]
    w = np.broadcast_to(w[:, None], (nl, 8, 16, n // 16)).reshape(nl, P, n // 16)
    return np.ascontiguousarray(w.transpose(1, 0, 2).astype(np.int16))


def _bucket_core(sidx: np.ndarray, nt: int):
    """Bucket one core's [bc, NS] int64 indices.

    Returns idxa [128, NSEG, npair//16] i16, idxf [128, 2] i32,
    idxb [128, nt, NS*TB//16] i16.
    """
    bc = sidx.shape[0]
    npair = nt * S
    b_of = np.repeat(np.arange(bc, dtype=np.int64), NS)
    k_of = np.tile(np.arange(NS, dtype=np.int64), bc)
    g = sidx.reshape(-1)                      # [bc*NS] global row
    t_of = b_of // TB
    w_of = g >> 15
    loc = g & 32767
    key = w_of * nt + t_of                    # bucket id, 0..NSEG*nt-1
    order = np.argsort(key, kind="stable")
    key_s = key[order]
    counts = np.bincount(key_s, minlength=NSEG * nt)
    starts = np.concatenate([[0], np.cumsum(counts)[:-1]])
    c = np.arange(bc * NS, dtype=np.int64) - starts[key_s]  # slot within bucket

    ok = c < S
    idxa = np.zeros((NSEG, npair), dtype=np.int64)
    w_s, t_s = w_of[order], t_of[order]
    idxa[w_s[ok], t_s[ok] * S + c[ok]] = loc[order][ok]
    # staging position: bucket (w,t) slot c=j*128+p -> row w*S + p*SJ + j
    pos_ok = w_s[ok] * S + (c[ok] % P) * SJ + c[ok] // P

    pos = np.zeros(bc * NS, dtype=np.int64)
    pos[order[ok]] = pos_ok
    ov = order[~ok]                            # overflow pair ids, sorted order
    nov = ov.shape[0]
    assert nov <= FIX, f"fixup overflow: {nov} > {FIX}"
    idxf = np.zeros((P,), dtype=np.int64)
    idxf[:nov] = g[ov]
    pos[ov] = NSEG * S + np.arange(nov)

    # pass B lists: column (k*TB + b%TB) of window t reads pos(b, k)
    idxb = np.zeros((nt, NS * TB), dtype=np.int64)
    idxb[t_of, k_of * TB + b_of % TB] = pos

    idxf2 = np.zeros((P, 2), dtype=np.int32)
    idxf2[:, 0] = idxf
    return _pack16(idxa), idxf2, _pack16(idxb)


def make_core_inputs(inp: dict, bc: int) -> list[dict]:
    """Shard x across cores; weights/table replicated; host-built indices."""
    shared = prep_weights(inp)
    x = np.asarray(inp["x"])
    nt = bc // TB
    in_maps = []
    for cidx in range(NCORES):
        xs = x[cidx * bc : (cidx + 1) * bc]
        sidx = xs[:, ND:].astype(np.int64) % VOCAB
        idxa, idxf, idxb = _bucket_core(sidx, nt)
        m = dict(shared)
        m["xdT"] = np.ascontiguousarray(xs[:, :ND].T, dtype=np.float32)
        m["idxa"] = idxa
        m["idxf"] = idxf
        m["idxb"] = idxb
        in_maps.append(m)
    return in_maps


_CACHE: dict = {}


def kernel(**inputs) -> np.ndarray:
    bc = BATCH // NCORES
    if "nc" not in _CACHE:
        _CACHE["nc"] = build_nc(bc)
    nc = _CACHE["nc"]
    in_maps = make_core_inputs(inputs, bc)
    res = run_bass_kernel_spmd(nc, in_maps, core_ids=list(range(NCORES)))
    outs = [res.results[c]["out"] for c in range(NCORES)]
    return np.concatenate(outs, axis=0).astype(np.float32)


# revision 10
# speedup vs baseline: 1.0825x; 1.0269x over previous
"""DLRM-ResNet (embedding_lookup) Trainium2 Bass kernel.

Strategy: data parallelism over the batch across 8 NeuronCores; each core
holds a full bf16 replica of the 2M x 128 table and processes 4096 rows.

The embedding gather is restructured as a 3-pass sort pipeline so the bulk
of the rows move through wide SWDGE dma_gather instructions (994ns fixed
overhead amortized over thousands of descriptors) instead of 832 small
indirect DMAs:

  pass A  (64 instrs): for each 32K-row table segment, gather that
          segment's rows for the whole core batch (host pre-buckets
          indices by (segment, batch-tile), int16 local ids, buckets
          padded to S=256 slots with dummy index 0) into an SBUF bounce.
  copy    (64 instrs): strided HWDGE copy bounce -> HBM staging laid out
          window-major: window t (batch tile of 512) owns a contiguous
          16512-row region; bucket (w,t) slot c=j*128+p lands at row
          w*256 + p*2 + j (copy-stream order), fixup tail at 16384.
  fixup   (1 indirect DMA): bucket-overflow pairs (rare) gathered with
          full-reach int32 indices, replicated into each window's tail.
  pass B  (8 instrs): per window, one transpose-mode dma_gather pulls the
          window's 26*512 rows from staging (int16 positions, host
          computed) directly into feature-major ze [128d, 26k*512b] bf16.

MLP per batch tile of 512 (feature-major, batch on free dim), as before:
  bottom MLP on host-pre-transposed dense f32 (f32r matmuls), top MLP
  layer 0 accumulates f32r h-part + 26 bf16 ze chunks in PSUM, relu+bias
  on ACT, residuals on DVE. Output [4096,1] f32, concatenated on host.
"""

import numpy as np
import ml_dtypes

import concourse.bass as bass
import concourse.bacc as bacc
import concourse.mybir as mybir
import concourse.tile as tile
from concourse.bass_utils import run_bass_kernel_spmd

F32 = mybir.dt.float32
F32R = mybir.dt.float32r
BF16 = mybir.dt.bfloat16
I16 = mybir.dt.int16
I32 = mybir.dt.int32

VOCAB = 2097152
D = 128          # embedding dim
NS = 26          # sparse features
ND = 13          # dense features
BATCH = 32768
NCORES = 8
P = 128

SEG = 32768              # table rows per segment (int16 reach)
NSEG = VOCAB // SEG      # 64
TB = 512                 # batch tile / window size
S = 256                  # bucket slots per (segment, window)
SJ = S // P              # 2
FIX = 128                # fixup slots per window
WROWS = NSEG * S + FIX   # 16512 staging rows per window

AF = mybir.ActivationFunctionType
ALU = mybir.AluOpType


def build_nc(bc: int):
    """Per-core program for a batch slice of `bc` rows (bc % TB == 0)."""
    nt = bc // TB            # windows / batch tiles
    WR = nt // 2             # windows per pass-A round
    npair = WR * S           # pass A num_idxs per (round, segment)

    nc = bacc.Bacc(
        "TRN2", target_bir_lowering=False, debug=False, num_devices=NCORES,
        dynamic_dma_scratch_size=32768,
    )

    xdT = nc.dram_tensor("xdT", [ND, bc], F32R, kind="ExternalInput")
    tab = nc.dram_tensor("tab", [VOCAB, D], BF16, kind="ExternalInput")
    idxa = nc.dram_tensor("idxa", [P, 2 * NSEG, (WR * S) // 16], I16, kind="ExternalInput")
    idxf = nc.dram_tensor("idxf", [P, 2], I32, kind="ExternalInput")
    idxb = nc.dram_tensor("idxb", [P, nt, (NS * TB) // 16], I16, kind="ExternalInput")
    wb0 = nc.dram_tensor("wb0", [ND, 256], F32R, kind="ExternalInput")
    wb12 = nc.dram_tensor("wb12", [P, 2, 2, 256], F32R, kind="ExternalInput")
    bbias = nc.dram_tensor("bbias", [P, 3, 2, 1], F32, kind="ExternalInput")
    w0h = nc.dram_tensor("w0h", [P, 2, 256], F32R, kind="ExternalInput")
    w0e = nc.dram_tensor("w0e", [P, NS, 256], BF16, kind="ExternalInput")
    wt123 = nc.dram_tensor("wt123", [P, 3, 2, 256], F32R, kind="ExternalInput")
    tbias = nc.dram_tensor("tbias", [P, 4, 2, 1], F32, kind="ExternalInput")
    w4 = nc.dram_tensor("w4", [P, 2, 1], F32R, kind="ExternalInput")
    tb4 = nc.dram_tensor("tb4", [1, 1], F32, kind="ExternalInput")
    out = nc.dram_tensor("out", [bc, 1], F32, kind="ExternalOutput")

    with tile.TileContext(nc) as tc:
        with (
            tc.tile_pool(name="stg", space="DRAM", bufs=1) as stgp,
            tc.tile_pool(name="wp", bufs=1) as wp,
            tc.tile_pool(name="bn", bufs=2) as bnp,
            tc.tile_pool(name="ze", bufs=2) as zep,
            tc.tile_pool(name="io", bufs=2) as io,
            tc.tile_pool(name="act", bufs=2) as actp,
            tc.tile_pool(name="psm", bufs=4, space="PSUM") as psm_pool,
            tc.tile_pool(name="pso", bufs=2, space="PSUM") as pso_pool,
        ):
            staging = stgp.tile([nt, WROWS, D], BF16)

            # ---- weight + index loads ----
            wb0_t = wp.tile([ND, 256], F32R)
            nc.sync.dma_start(wb0_t[:], wb0[:])
            wb12_t = wp.tile([P, 2, 2, 256], F32R)
            nc.sync.dma_start(wb12_t[:], wb12[:])
            bb_t = wp.tile([P, 3, 2, 1], F32)
            nc.sync.dma_start(bb_t[:], bbias[:])
            w0h_t = wp.tile([P, 2, 256], F32R)
            nc.sync.dma_start(w0h_t[:], w0h[:])
            w0e_t = wp.tile([P, NS, 256], BF16)
            nc.sync.dma_start(w0e_t[:], w0e[:])
            wt123_t = wp.tile([P, 3, 2, 256], F32R)
            nc.sync.dma_start(wt123_t[:], wt123[:])
            tb_t = wp.tile([P, 4, 2, 1], F32)
            nc.sync.dma_start(tb_t[:], tbias[:])
            w4_t = wp.tile([P, 2, 1], F32R)
            nc.sync.dma_start(w4_t[:], w4[:])
            tb4_t = wp.tile([1, 1], F32)
            nc.sync.dma_start(tb4_t[:], tb4[:])

            ib = wp.tile([P, nt, (NS * TB) // 16], I16)
            nc.sync.dma_start(ib[:], idxb[:])
            ift = wp.tile([P, 2], I32)
            nc.sync.dma_start(ift[:], idxf[:])

            # ---- fixup first: full-reach indirect gather of overflow rows ----
            fb = wp.tile([P, 2, D], BF16)
            nc.gpsimd.indirect_dma_start(
                out=fb[:, 0, :],
                out_offset=None,
                in_=tab[:],
                in_offset=bass.IndirectOffsetOnAxis(ap=ift[:, 0:1], axis=0),
            )
            for t in range(nt):
                nc.sync.dma_start(
                    staging[t, NSEG * S : NSEG * S + FIX, :], fb[:, 0, :]
                )

            # ---- pass A (two rounds) + pass B + MLP, interleaved ----
            IAC = 8  # segments per index-chunk load

            def passA(r, w):
                if w % IAC == 0:
                    ia = io.tile([P, IAC, npair // 16], I16, tag=f"ia{(w // IAC) % 2}")
                    nc.sync.dma_start(ia[:], idxa[:, r * NSEG + w : r * NSEG + w + IAC])
                    passA.ia = ia
                bounce = bnp.tile([P, WR * SJ, D], BF16, tag=f"bn{w % 2}")
                nc.gpsimd.dma_gather(
                    out_ap=bounce[:],
                    in_ap=tab[w * SEG : (w + 1) * SEG, :],
                    idxs_ap=passA.ia[:, w % IAC],
                    num_idxs=npair,
                    num_idxs_reg=npair,
                    elem_size=D,
                    single_packet=False,
                )
                src = bounce[:].rearrange("p (t j) e -> p t (j e)", t=WR, j=SJ)
                dst = staging[r * WR : (r + 1) * WR, w * S : (w + 1) * S, :]
                dst = dst.rearrange("t (p j) e -> p t (j e)", p=P, j=SJ)
                nc.sync.dma_start(dst, src)

            def passB(t):
                ze = zep.tile([P, 1, NS * TB], BF16, tag="ze")
                nc.gpsimd.dma_gather(
                    out_ap=ze[:],
                    in_ap=staging[t],
                    idxs_ap=ib[:, t],
                    num_idxs=NS * TB,
                    num_idxs_reg=NS * TB,
                    elem_size=D,
                    transpose=True,
                    single_packet=False,
                )
                return ze

            for w in range(NSEG):
                passA(0, w)

            # round 1 gathers interleaved between the early pass-B gathers
            r1_chunks = [list(range(0, 24)), list(range(24, 48)),
                         list(range(48, NSEG))]

            zes = {}
            for t in range(nt):
                c0 = t * TB

                if t < len(r1_chunks):
                    zes[t] = passB(t)
                    for w in r1_chunks[t]:
                        passA(1, w)
                elif t not in zes:
                    zes[t] = passB(t)
                # prefetch next tile's pass B right away
                if t + 1 < nt and t + 1 not in zes and t + 1 >= len(r1_chunks):
                    zes[t + 1] = passB(t + 1)
                ze = zes.pop(t)

                dT = io.tile([ND, TB], F32R, tag="dT")
                nc.sync.dma_start(dT[:], xdT[:, c0 : c0 + TB])

                # ---- bottom MLP (feature-major) ----
                h1 = actp.tile([P, 2, TB], F32, tag="hA")
                for m in range(2):
                    ps = psm_pool.tile([P, TB], F32, tag="psm")
                    nc.tensor.matmul(
                        ps[:],
                        wb0_t[:, m * P : (m + 1) * P],
                        dT[:],
                        start=True,
                        stop=True,
                    )
                    nc.scalar.activation(
                        h1[:, m, :], ps[:], AF.Relu, bias=bb_t[:, 0, m, :]
                    )
                hprev = h1
                for l in range(2):
                    hn = actp.tile([P, 2, TB], F32, tag=f"h{'BA'[l]}")
                    for m in range(2):
                        ps = psm_pool.tile([P, TB], F32, tag="psm")
                        for k in range(2):
                            nc.tensor.matmul(
                                ps[:],
                                wb12_t[:, l, k, m * P : (m + 1) * P],
                                hprev[:, k, :].bitcast(F32R),
                                start=(k == 0),
                                stop=(k == 1),
                            )
                        nc.scalar.activation(
                            hn[:, m, :], ps[:], AF.Relu, bias=bb_t[:, l + 1, m, :]
                        )
                        nc.vector.tensor_tensor(
                            hn[:, m, :], hn[:, m, :], hprev[:, m, :], op=ALU.add
                        )
                    hprev = hn

                # ---- top MLP layer 0: h-part (f32r) + 26 bf16 ze chunks ----
                z1 = actp.tile([P, 2, TB], F32, tag="zA")
                for m in range(2):
                    ps = psm_pool.tile([P, TB], F32, tag="psm")
                    for k in range(2):
                        nc.tensor.matmul(
                            ps[:],
                            w0h_t[:, k, m * P : (m + 1) * P],
                            hprev[:, k, :].bitcast(F32R),
                            start=(k == 0),
                            stop=False,
                        )
                    for k in range(NS):
                        nc.tensor.matmul(
                            ps[:],
                            w0e_t[:, k, m * P : (m + 1) * P],
                            ze[:, 0, k * TB : (k + 1) * TB],
                            start=False,
                            stop=(k == NS - 1),
                        )
                    nc.scalar.activation(
                        z1[:, m, :], ps[:], AF.Relu, bias=tb_t[:, 0, m, :]
                    )

                # ---- top residual layers 1..3 ----
                zprev = z1
                for l in range(3):
                    zn = actp.tile([P, 2, TB], F32, tag=f"z{'BAB'[l]}")
                    for m in range(2):
                        ps = psm_pool.tile([P, TB], F32, tag="psm")
                        for k in range(2):
                            nc.tensor.matmul(
                                ps[:],
                                wt123_t[:, l, k, m * P : (m + 1) * P],
                                zprev[:, k, :].bitcast(F32R),
                                start=(k == 0),
                                stop=(k == 1),
                            )
                        nc.scalar.activation(
                            zn[:, m, :], ps[:], AF.Relu, bias=tb_t[:, l + 1, m, :]
                        )
                        nc.vector.tensor_tensor(
                            zn[:, m, :], zn[:, m, :], zprev[:, m, :], op=ALU.add
                        )
                    zprev = zn

                # ---- final linear [256 -> 1] ----
                po = pso_pool.tile([1, TB], F32, tag="pso")
                for k in range(2):
                    nc.tensor.matmul(
                        po[:],
                        w4_t[:, k, :],
                        zprev[:, k, :].bitcast(F32R),
                        start=(k == 0),
                        stop=(k == 1),
                    )
                ot = io.tile([1, TB], F32, tag="ot")
                nc.scalar.activation(
                    ot[:, :], po[:], AF.Identity, bias=tb4_t[:]
                )
                nc.sync.dma_start(out[c0 : c0 + TB, :], ot[:])

    nc.compile()
    return nc


def prep_weights(inp: dict) -> dict:
    """Host-side layout prep shared by all cores (all partition-major)."""
    f32 = np.float32
    bw0, bw1, bw2 = inp["bw0"], inp["bw1"], inp["bw2"]
    tw = [inp[f"tw{i}"] for i in range(5)]

    wb12 = np.stack(
        [w.T.reshape(2, P, 256).transpose(1, 0, 2) for w in (bw1, bw2)], axis=1
    )  # [128, 2(layer), 2(k), 256]
    bbias = np.stack(
        [inp[f"bb{i}"].reshape(2, P).T for i in range(3)], axis=1
    ).reshape(P, 3, 2, 1)

    t0T = tw[0].T  # [3584, 256]
    w0h = t0T[:256].reshape(2, P, 256).transpose(1, 0, 2)  # [128, 2, 256]
    w0e = (
        t0T[256:]
        .reshape(NS, P, 256)
        .transpose(1, 0, 2)
        .astype(ml_dtypes.bfloat16)
    )  # [128, 26, 256]
    wt123 = np.stack(
        [w.T.reshape(2, P, 256).transpose(1, 0, 2) for w in tw[1:4]], axis=1
    )  # [128, 3(layer), 2(k), 256]
    tbias = np.stack(
        [inp[f"tb{i}"].reshape(2, P).T for i in range(4)], axis=1
    ).reshape(P, 4, 2, 1)
    w4 = tw[4].T.reshape(2, P, 1).transpose(1, 0, 2)  # [128, 2, 1]
    tb4 = inp["tb4"].reshape(1, 1)

    tab = np.concatenate(
        [inp["emb0"], inp["emb1"], inp["emb2"], inp["emb3"]], axis=0
    ).astype(ml_dtypes.bfloat16)

    return {
        "wb0": np.ascontiguousarray(bw0.T, dtype=f32),
        "wb12": np.ascontiguousarray(wb12, dtype=f32),
        "bbias": np.ascontiguousarray(bbias, dtype=f32),
        "w0h": np.ascontiguousarray(w0h, dtype=f32),
        "w0e": np.ascontiguousarray(w0e),
        "wt123": np.ascontiguousarray(wt123.reshape(P, 3, 2, 256), dtype=f32),
        "tbias": np.ascontiguousarray(tbias, dtype=f32),
        "w4": np.ascontiguousarray(w4, dtype=f32),
        "tb4": np.ascontiguousarray(tb4, dtype=f32),
        "tab": np.ascontiguousarray(tab),
    }


def _pack16(lists: np.ndarray) -> np.ndarray:
    """[n_lists, n] int -> [128, n_lists, n//16] wrap-16 int16, 8x replicated."""
    nl, n = lists.shape
    w = lists.reshape(nl, n // 16, 16).transpose(0, 2, 1)  # [nl, 16, n//16]
    w = np.broadcast_to(w[:, None], (nl, 8, 16, n // 16)).reshape(nl, P, n // 16)
    return np.ascontiguousarray(w.transpose(1, 0, 2).astype(np.int16))


def _bucket_core(sidx: np.ndarray, nt: int):
    """Bucket one core's [bc, NS] int64 indices.

    Returns idxa [128, NSEG, npair//16] i16, idxf [128, 2] i32,
    idxb [128, nt, NS*TB//16] i16.
    """
    bc = sidx.shape[0]
    npair = nt * S
    b_of = np.repeat(np.arange(bc, dtype=np.int64), NS)
    k_of = np.tile(np.arange(NS, dtype=np.int64), bc)
    g = sidx.reshape(-1)                      # [bc*NS] global row
    t_of = b_of // TB
    w_of = g >> 15
    loc = g & 32767
    key = w_of * nt + t_of                    # bucket id, 0..NSEG*nt-1
    order = np.argsort(key, kind="stable")
    key_s = key[order]
    counts = np.bincount(key_s, minlength=NSEG * nt)
    starts = np.concatenate([[0], np.cumsum(counts)[:-1]])
    c = np.arange(bc * NS, dtype=np.int64) - starts[key_s]  # slot within bucket

    ok = c < S
    idxa = np.zeros((NSEG, npair), dtype=np.int64)
    w_s, t_s = w_of[order], t_of[order]
    idxa[w_s[ok], t_s[ok] * S + c[ok]] = loc[order][ok]
    # staging position: bucket (w,t) slot c=j*128+p -> row w*S + p*SJ + j
    pos_ok = w_s[ok] * S + (c[ok] % P) * SJ + c[ok] // P

    pos = np.zeros(bc * NS, dtype=np.int64)
    pos[order[ok]] = pos_ok
    ov = order[~ok]                            # overflow pair ids, sorted order
    nov = ov.shape[0]
    assert nov <= FIX, f"fixup overflow: {nov} > {FIX}"
    idxf = np.zeros((P,), dtype=np.int64)
    idxf[:nov] = g[ov]
    pos[ov] = NSEG * S + np.arange(nov)

    # pass B lists: column (k*TB + b%TB) of window t reads pos(b, k)
    idxb = np.zeros((nt, NS * TB), dtype=np.int64)
    idxb[t_of, k_of * TB + b_of % TB] = pos

    idxf2 = np.zeros((P, 2), dtype=np.int32)
    idxf2[:, 0] = idxf
    return _pack16(idxa), idxf2, _pack16(idxb)


def make_core_inputs(inp: dict, bc: int) -> list[dict]:
    """Shard x across cores; weights/table replicated; host-built indices."""
    shared = prep_weights(inp)
    x = np.asarray(inp["x"])
    nt = bc // TB
    in_maps = []
    for cidx in range(NCORES):
        xs = x[cidx * bc : (cidx + 1) * bc]
        sidx = xs[:, ND:].astype(np.int64) % VOCAB
        idxa, idxf, idxb = _bucket_core(sidx, nt)
        m = dict(shared)
        m["xdT"] = np.ascontiguousarray(xs[:, :ND].T, dtype=np.float32)
        m["idxa"] = idxa
        m["idxf"] = idxf
        m["idxb"] = idxb
        in_maps.append(m)
    return in_maps


_CACHE: dict = {}


def kernel(**inputs) -> np.ndarray:
    bc = BATCH // NCORES
    if "nc" not in _CACHE:
        _CACHE["nc"] = build_nc(bc)
    nc = _CACHE["nc"]
    in_maps = make_core_inputs(inputs, bc)
    res = run_bass_kernel_spmd(nc, in_maps, core_ids=list(range(NCORES)))
    outs = [res.results[c]["out"] for c in range(NCORES)]
    return np.concatenate(outs, axis=0).astype(np.float32)
